# revision 12
# baseline (speedup 1.0000x reference)
"""LocallyConnected2D (B=16, 32x32, CIN=COUT=64, 3x3, pad=1) on 8 TRN2 NeuronCores.

Shard the 32 output rows across 8 cores (4 rows each); all tensors fp16 on
device (fp32 PSUM accumulate), fp32 finish on host.

Tap pairing (k = 3*di + dj): pairs (0,3), (1,4), (2,5) put taps with the SAME
column shift dj=t on the two partition halves (rows r and r+1), so the lhsT
for a K=128 matmul is just two consecutive padded x rows stacked — one
contiguous DMA, no shifted copies. Row r+2's taps 6,7,8 run as three K=64
solo matmuls. Per pixel: 6 PSUM-accumulating matmuls, M=16 (batch), N=64
(cout); 4 pixels run concurrently in the PE array via column tile_position.

out[b,i,j,o] = sum_{c,k} x_pad[b, i+di, j+dj, c] * W[o,c,i,j,k].

Host layouts (per core c, local row r, i = 4c+r, j = 4g+jj, m = half):
  w_pairs [4, 128, 6144]: [64m+cin, j*192 + t*64 + o] = W[o, cin, i, j, t+3m]
  w_solo  [4, 128, 3072]: [64(j%2)+cin, (j//2)*192 + t*64 + o] = W[o, cin, i, j, 6+t]
  xt      [384, 512]:     [rin*64+cin, j*16+b] = x_pad[b, 4c+rin, j, cin]
  out     [4, 16, 2048] fp16: [jj, b, r*512 + g*64 + o] = out[b, i, j, o]

x panels V(k) [128, 544], k=0..4: partitions = x rows (k, k+1), padded col c
stored at (c+1)*16 (memset zero borders for c=-1, 32); S5 [64,544] = row 5.
Pair t lhsT = V(r)[:, (j+t)*16:+16]. Solo t: row r+2 = V(r+2)[0:64] (j even,
r<3), V(r+1)[64:128] (j odd), S5[0:64] (j even, r=3).

PSUM: one [128, 512] bank per r accumulates all 8 column groups; 4 DVE
casts f32->fp16 per r into stage [128, 2048]; 4 output DMAs per core.
Weight DMAs ride the SP HWDGE ring; x/out DMAs ride the ACT ring.
"""

import numpy as np

B, IH, IW, CIN = 16, 32, 32, 64
COUT, OH, OW = 64, 32, 32
NCORES, RPC = 8, 4

_NC = None


def _build_nc(n_reps=1):
    import concourse.bacc as bacc
    import concourse.mybir as mybir
    import concourse.tile as tile

    f16 = mybir.dt.float16
    f32 = mybir.dt.float32
    nc = bacc.Bacc("TRN2", target_bir_lowering=False, debug=False)
    wp = nc.dram_tensor("w_pairs", [RPC, 128, 6144], f16, kind="ExternalInput")
    wso = nc.dram_tensor("w_solo", [RPC, 128, 3072], f16, kind="ExternalInput")
    xt = nc.dram_tensor("xt", [384, 512], f16, kind="ExternalInput")
    out = nc.dram_tensor("out", [4, 16, RPC * 512], f16, kind="ExternalOutput")
    wp_ap, wso_ap, xt_ap, out_ap = wp.ap(), wso.ap(), xt.ap(), out.ap()

    with tile.TileContext(nc) as tc:
        with (
            tc.tile_pool(name="wp", bufs=3) as wp_pool,
            tc.tile_pool(name="wso", bufs=3) as wso_pool,
            tc.tile_pool(name="vx", bufs=2) as vx_pool,
            tc.tile_pool(name="stage", bufs=2) as stage_pool,
            tc.tile_pool(name="psum", bufs=4, space="PSUM") as psum_pool,
        ):
            for rep in range(n_reps):
                # x panels: V(k) = padded x rows (k, k+1), col c at (c+1)*16
                vs = []
                for k in range(5):
                    v = vx_pool.tile([128, 544], f16, tag=f"v{k}")
                    nc.gpsimd.memset(v[:, 0:16], 0.0)
                    nc.gpsimd.memset(v[:, 528:544], 0.0)
                    nc.scalar.dma_start(v[:, 16:528], xt_ap[64 * k : 64 * k + 128])
                    vs.append(v)
                s5 = vx_pool.tile([64, 544], f16, tag="s5")
                nc.gpsimd.memset(s5[:, 0:16], 0.0)
                nc.gpsimd.memset(s5[:, 528:544], 0.0)
                nc.scalar.dma_start(s5[:, 16:528], xt_ap[320:384])

                stage = stage_pool.tile([128, 2048], f16, tag="stage")
                for r in range(RPC):
                    wp_t = wp_pool.tile([128, 6144], f16, tag="wp")
                    wso_t = wso_pool.tile([128, 3072], f16, tag="wso")
                    # split so the first column groups can start early;
                    # pair/solo weights ride different HWDGE rings
                    nc.sync.dma_start(wp_t[:, 0:3072], wp_ap[r][:, 0:3072])
                    nc.scalar.dma_start(wso_t[:, 0:1536], wso_ap[r][:, 0:1536])
                    nc.sync.dma_start(wp_t[:, 3072:6144], wp_ap[r][:, 3072:6144])
                    nc.scalar.dma_start(wso_t[:, 1536:3072], wso_ap[r][:, 1536:3072])

                    wp_v = wp_t[:].rearrange("p (j t o) -> p j t o", o=64, t=3)
                    wso_v = wso_t[:].rearrange("p (h t o) -> p h t o", o=64, t=3)

                    ps = psum_pool.tile([128, 512], f32, tag="ps")
                    for g in range(8):
                        for t in range(6):
                            for jj in range(4):
                                j = 4 * g + jj
                                if t < 3:
                                    lhsT = vs[r][:, (j + t) * 16 : (j + t + 1) * 16]
                                    rhs = wp_v[:, j, t, :]
                                    tp = (0, 32 * jj)
                                else:
                                    dj = t - 3
                                    off = (j + dj) * 16
                                    if j % 2 == 1:
                                        lhsT = vs[r + 1][64:128, off : off + 16]
                                        rhs = wso_v[64:128, j // 2, dj, :]
                                        tp = (64, 32 * jj)
                                    else:
                                        src = s5 if r == 3 else vs[r + 2]
                                        lhsT = src[0:64, off : off + 16]
                                        rhs = wso_v[0:64, j // 2, dj, :]
                                        tp = (0, 32 * jj)
                                nc.tensor.matmul(
                                    ps[32 * jj : 32 * jj + 16, 64 * g : 64 * g + 64],
                                    lhsT,
                                    rhs,
                                    start=(t == 0),
                                    stop=(t == 5),
                                    tile_position=tp,
                                    skip_group_check=True,
                                )
                    for jj in range(4):
                        nc.vector.tensor_copy(
                            stage[32 * jj : 32 * jj + 16, r * 512 : (r + 1) * 512],
                            ps[32 * jj : 32 * jj + 16, :],
                        )
                        nc.scalar.dma_start(
                            out_ap[jj][:, r * 512 : (r + 1) * 512],
                            stage[32 * jj : 32 * jj + 16, r * 512 : (r + 1) * 512],
                        )
    nc.compile()
    return nc


def _repack_inputs(x, weight):
    x = np.asarray(x, dtype=np.float32)
    weight = np.asarray(weight, dtype=np.float32)

    wt = np.ascontiguousarray(weight.transpose(2, 1, 0, 3, 4))  # [i, c, o, j, k]
    a = wt[..., :6].reshape(OH, CIN, COUT, OW, 2, 3)  # [i,c,o,j,m,t]
    wpair = (
        np.ascontiguousarray(a.transpose(0, 4, 1, 3, 5, 2))  # [i,m,c,j,t,o]
        .reshape(OH, 128, 6144)
        .astype(np.float16)
    )
    b6 = wt[..., 6:9].reshape(OH, CIN, COUT, 16, 2, 3)  # [i,c,o,jh,jp,t]
    wsolo = (
        np.ascontiguousarray(b6.transpose(0, 4, 1, 3, 5, 2))  # [i,jp,c,jh,t,o]
        .reshape(OH, 128, 3072)
        .astype(np.float16)
    )

    xpad = np.zeros((IH + 2, CIN, IW, B), dtype=np.float16)
    xpad[1:33] = x.transpose(1, 3, 2, 0)  # [ih, c, j, b]

    in_maps = []
    for c in range(NCORES):
        in_maps.append(
            {
                "w_pairs": np.ascontiguousarray(wpair[c * RPC : (c + 1) * RPC]),
                "w_solo": np.ascontiguousarray(wsolo[c * RPC : (c + 1) * RPC]),
                "xt": np.ascontiguousarray(
                    xpad[c * RPC : c * RPC + RPC + 2].reshape(384, 512)
                ),
            }
        )
    return in_maps


def _get_nc():
    global _NC
    if _NC is None:
        _NC = _build_nc()
    return _NC


def run_spmd(in_maps, **kwargs):
    from concourse.bass_utils import run_bass_kernel_spmd

    return run_bass_kernel_spmd(
        _get_nc(), in_maps, core_ids=list(range(NCORES)), **kwargs
    )


def kernel(x, weight, bias, _results=None):
    if _results is None:
        _results = run_spmd(_repack_inputs(x, weight)).results
    arr = np.stack([r["out"] for r in _results]).astype(np.float32)
    arr = arr.reshape(NCORES, 4, 16, RPC, 8, 64)
    # arr: [core, jj, b, r, g, o] -> out[b, 4c+r, 4g+jj, o]
    out = arr.transpose(2, 0, 3, 4, 1, 5).reshape(B, OH, OW, COUT)
    return out + np.asarray(bias, dtype=np.float32)[None]


# revision 13
# speedup vs baseline: 1.2891x; 1.2891x over previous
"""LocallyConnected2D (B=16, 32x32, CIN=COUT=64, 3x3, pad=1) on 8 TRN2 NeuronCores.

Shard the 32 output rows across 8 cores (4 rows each); all tensors fp16 on
device (fp32 PSUM accumulate), fp32 finish on host.

Tap pairing (k = 3*di + dj): pairs (0,3), (1,4), (2,5) put taps with the SAME
column shift dj=t on the two partition halves (rows r and r+1), so the lhsT
for a K=128 matmul is just two consecutive padded x rows stacked — one
contiguous DMA, no shifted copies. Row r+2's taps 6,7,8 run as three K=64
solo matmuls. Per pixel: 6 PSUM-accumulating matmuls, M=16 (batch), N=64
(cout); 4 pixels run concurrently in the PE array via column tile_position.

out[b,i,j,o] = sum_{c,k} x_pad[b, i+di, j+dj, c] * W[o,c,i,j,k].

Host layouts (per core c, local row r, i = 4c+r, j = 4g+jj, m = half):
  w_pairs [4, 128, 6144]: [64m+cin, j*192 + t*64 + o] = W[o, cin, i, j, t+3m]
  w_solo  [4, 128, 3072]: [64(j%2)+cin, (j//2)*192 + t*64 + o] = W[o, cin, i, j, 6+t]
  xt      [384, 512]:     [rin*64+cin, j*16+b] = x_pad[b, 4c+rin, j, cin]
  out     [4, 16, 2048] fp16: [jj, b, r*512 + g*64 + o] = out[b, i, j, o]

x panels V(k) [128, 544], k=0..4: partitions = x rows (k, k+1), padded col c
stored at (c+1)*16 (memset zero borders for c=-1, 32); S5 [64,544] = row 5.
Pair t lhsT = V(r)[:, (j+t)*16:+16]. Solo t: row r+2 = V(r+2)[0:64] (j even,
r<3), V(r+1)[64:128] (j odd), S5[0:64] (j even, r=3).

PSUM: one [128, 512] bank per r accumulates all 8 column groups; 4 DVE
casts f32->fp16 per r into stage [128, 2048]; 4 output DMAs per core.
Weight DMAs ride the SP HWDGE ring; x/out DMAs ride the ACT ring.
"""

import os
import numpy as np

PROBE = os.environ.get("KPROBE", "")

B, IH, IW, CIN = 16, 32, 32, 64
COUT, OH, OW = 64, 32, 32
NCORES, RPC = 8, 4

_NC = None


def _build_nc(n_reps=1):
    import concourse.bacc as bacc
    import concourse.mybir as mybir
    import concourse.tile as tile

    f16 = mybir.dt.float16
    f32 = mybir.dt.float32
    nc = bacc.Bacc("TRN2", target_bir_lowering=False, debug=False)
    wp = nc.dram_tensor("w_pairs", [RPC, 128, 6144], f16, kind="ExternalInput")
    wso = nc.dram_tensor("w_solo", [RPC, 128, 3072], f16, kind="ExternalInput")
    xt = nc.dram_tensor("xt", [384, 512], f16, kind="ExternalInput")
    out = nc.dram_tensor("out", [4, 16, RPC * 512], f16, kind="ExternalOutput")
    wp_ap, wso_ap, xt_ap, out_ap = wp.ap(), wso.ap(), xt.ap(), out.ap()

    with tile.TileContext(nc) as tc:
        with (
            tc.tile_pool(name="wp", bufs=3) as wp_pool,
            tc.tile_pool(name="wso", bufs=3) as wso_pool,
            tc.tile_pool(name="vx", bufs=2) as vx_pool,
            tc.tile_pool(name="stage", bufs=2) as stage_pool,
            tc.tile_pool(name="psum", bufs=4, space="PSUM") as psum_pool,
        ):
            for rep in range(n_reps):
                # x panels: V(k) = padded x rows (k, k+1), col c at (c+1)*16
                vs = []
                for k in range(5):
                    v = vx_pool.tile([128, 544], f16, tag=f"v{k}")
                    nc.gpsimd.memset(v[:, 0:16], 0.0)
                    nc.gpsimd.memset(v[:, 528:544], 0.0)
                    nc.scalar.dma_start(v[:, 16:528], xt_ap[64 * k : 64 * k + 128])
                    vs.append(v)
                s5 = vx_pool.tile([64, 544], f16, tag="s5")
                nc.gpsimd.memset(s5[:, 0:16], 0.0)
                nc.gpsimd.memset(s5[:, 528:544], 0.0)
                nc.scalar.dma_start(s5[:, 16:528], xt_ap[320:384])

                stage = stage_pool.tile([128, 2048], f16, tag="stage")
                for r in range(RPC):
                    wp_t = wp_pool.tile([128, 6144], f16, tag="wp")
                    wso_t = wso_pool.tile([128, 3072], f16, tag="wso")
                    # split so the first column groups can start early;
                    # pair/solo weights ride different HWDGE rings
                    nc.sync.dma_start(wp_t[:, 0:3072], wp_ap[r][:, 0:3072])
                    nc.scalar.dma_start(wso_t[:, 0:1536], wso_ap[r][:, 0:1536])
                    if PROBE != "halfdma":
                        nc.sync.dma_start(wp_t[:, 3072:6144], wp_ap[r][:, 3072:6144])
                        nc.scalar.dma_start(wso_t[:, 1536:3072], wso_ap[r][:, 1536:3072])

                    wp_v = wp_t[:].rearrange("p (j t o) -> p j t o", o=64, t=3)
                    wso_v = wso_t[:].rearrange("p (h t o) -> p h t o", o=64, t=3)

                    ps = psum_pool.tile([128, 512], f32, tag="ps")
                    for g in range(8):
                        for t in range(6):
                            for jj in range(4):
                                j = 4 * g + jj
                                jw = 4 * (g % 4) + jj if PROBE == "halfdma" else j
                                if t < 3:
                                    lhsT = vs[r][:, (j + t) * 16 : (j + t + 1) * 16]
                                    rhs = wp_v[:, jw, t, :]
                                    tp = (0, 32 * jj)
                                else:
                                    dj = t - 3
                                    off = (j + dj) * 16
                                    if j % 2 == 1:
                                        lhsT = vs[r + 1][64:128, off : off + 16]
                                        rhs = wso_v[64:128, jw // 2, dj, :]
                                        tp = (64, 32 * jj)
                                    else:
                                        src = s5 if r == 3 else vs[r + 2]
                                        lhsT = src[0:64, off : off + 16]
                                        rhs = wso_v[0:64, jw // 2, dj, :]
                                        tp = (0, 32 * jj)
                                nc.tensor.matmul(
                                    ps[32 * jj : 32 * jj + 16, 64 * g : 64 * g + 64],
                                    lhsT,
                                    rhs,
                                    start=(t == 0),
                                    stop=(t == 5),
                                    tile_position=tp,
                                    skip_group_check=True,
                                )
                    for jj in range(4):
                        nc.vector.tensor_copy(
                            stage[32 * jj : 32 * jj + 16, r * 512 : (r + 1) * 512],
                            ps[32 * jj : 32 * jj + 16, :],
                        )
                        nc.scalar.dma_start(
                            out_ap[jj][:, r * 512 : (r + 1) * 512],
                            stage[32 * jj : 32 * jj + 16, r * 512 : (r + 1) * 512],
                        )
    nc.compile()
    return nc


def _repack_inputs(x, weight):
    x = np.asarray(x, dtype=np.float32)
    weight = np.asarray(weight, dtype=np.float32)

    wt = np.ascontiguousarray(weight.transpose(2, 1, 0, 3, 4))  # [i, c, o, j, k]
    a = wt[..., :6].reshape(OH, CIN, COUT, OW, 2, 3)  # [i,c,o,j,m,t]
    wpair = (
        np.ascontiguousarray(a.transpose(0, 4, 1, 3, 5, 2))  # [i,m,c,j,t,o]
        .reshape(OH, 128, 6144)
        .astype(np.float16)
    )
    b6 = wt[..., 6:9].reshape(OH, CIN, COUT, 16, 2, 3)  # [i,c,o,jh,jp,t]
    wsolo = (
        np.ascontiguousarray(b6.transpose(0, 4, 1, 3, 5, 2))  # [i,jp,c,jh,t,o]
        .reshape(OH, 128, 3072)
        .astype(np.float16)
    )

    xpad = np.zeros((IH + 2, CIN, IW, B), dtype=np.float16)
    xpad[1:33] = x.transpose(1, 3, 2, 0)  # [ih, c, j, b]

    in_maps = []
    for c in range(NCORES):
        in_maps.append(
            {
                "w_pairs": np.ascontiguousarray(wpair[c * RPC : (c + 1) * RPC]),
                "w_solo": np.ascontiguousarray(wsolo[c * RPC : (c + 1) * RPC]),
                "xt": np.ascontiguousarray(
                    xpad[c * RPC : c * RPC + RPC + 2].reshape(384, 512)
                ),
            }
        )
    return in_maps


def _get_nc():
    global _NC
    if _NC is None:
        _NC = _build_nc()
    return _NC


def run_spmd(in_maps, **kwargs):
    from concourse.bass_utils import run_bass_kernel_spmd

    return run_bass_kernel_spmd(
        _get_nc(), in_maps, core_ids=list(range(NCORES)), **kwargs
    )


def kernel(x, weight, bias, _results=None):
    if _results is None:
        _results = run_spmd(_repack_inputs(x, weight)).results
    arr = np.stack([r["out"] for r in _results]).astype(np.float32)
    arr = arr.reshape(NCORES, 4, 16, RPC, 8, 64)
    # arr: [core, jj, b, r, g, o] -> out[b, 4c+r, 4g+jj, o]
    out = arr.transpose(2, 0, 3, 4, 1, 5).reshape(B, OH, OW, COUT)
    return out + np.asarray(bias, dtype=np.float32)[None]
